# revision 3
# baseline (speedup 1.0000x reference)
"""Trainium2 Bass kernel for InvariantMessagePassingTP.

out[n, lm, c] = sum_{e: recv[e]=n} edge_attrs[e,lm] * tp_weights[e,l(lm),c]
                * node_feats[recv[e], c]

Strategy (8 NeuronCores, SPMD):
  receiver_list is sorted, so edges for a node are contiguous. Nodes are
  split into 128-node windows; each core owns 25 consecutive windows
  (196 real windows over 25000 nodes). Per window, edges are processed in
  128-edge tiles (edges on SBUF partitions):
    U   = W * F            (DVE, bf16 2x, F broadcast over l)
    msg = A * U[l(lm)]     (DVE l2/l3 + GPSIMD l0/l1, bf16 2x via the
                            host-side A-duplication trick: A2[e,lm,2])
    out_w += S^T @ msg     (PE: S = host-built one-hot [edge, node-in-window],
                            2 matmuls N=512 bf16, accumulated in PSUM over
                            all tiles of the window)
  then ACT copies PSUM->SBUF and the window rows go to DRAM. Output rows are
  disjoint across cores - no collective needed.

The per-window packed buffer is block-major per partition (= per edge slot):
  [ W: T*256 | F: T*64 | A2_l0: T*2 | A2_l1: T*6 | A2_l2: T*10 |
    A2_l3: T*14 | S: T*128 ]   (bf16 elems, T = tiles in window)
so that every engine operand collapses to <=3 free dims.
"""

import sys

sys.path.insert(0, "/opt/trn_rl_repo")

import numpy as np
import ml_dtypes

import concourse.bass as bass
import concourse.bacc as bacc
import concourse.tile as tile
from concourse import mybir
from concourse.bass_utils import run_bass_kernel_spmd

NPBF = ml_dtypes.bfloat16
BF16 = mybir.dt.bfloat16
F32 = mybir.dt.float32

NNODES = 25000
NEDGES = 400000
NCHAN = 64
N_CORES = 8
WIN = 128          # nodes per window
W_PER_CORE = 25    # window slots per core (8*25*128 = 25600 >= 25000)
TB = 480           # bf16 elems per tile per partition in the packed buffer
MSG_B = 4          # tiles per DVE/GPSIMD op batch

L_OF_LM = np.array([0, 1, 1, 1, 2, 2, 2, 2, 2, 3, 3, 3, 3, 3, 3, 3], np.int64)
L_GROUPS = [(0, 1), (1, 3), (4, 5), (9, 7)]  # (lm_start, m_l) for l=0..3

_PROGRAM_CACHE = {}


def _block_offsets(T):
    """bf16-elem offsets of each block within the [128, T*TB] window buffer."""
    off = {}
    o = 0
    off["W"] = o; o += T * 256
    off["F"] = o; o += T * 64
    for g, (_, m_l) in enumerate(L_GROUPS):
        off[f"A{g}"] = o; o += T * m_l * 2
    off["S"] = o; o += T * 128
    assert o == T * TB
    return off


def _build_schedule(receiver_list):
    recv = np.asarray(receiver_list).astype(np.int64)
    n_win_total = N_CORES * W_PER_CORE
    bounds = np.searchsorted(recv, np.arange(0, (n_win_total + 1) * WIN, WIN))
    counts = bounds[1:] - bounds[:-1]
    per_core = counts.reshape(N_CORES, W_PER_CORE)
    t_slot = np.maximum(1, np.ceil(per_core / 128.0).astype(np.int64).max(axis=0))
    return bounds, t_slot


def _pack_inputs(node_feats, edge_attrs, tp_weights, receiver_list, bounds, t_slot):
    recv = np.asarray(receiver_list).astype(np.int64)
    w_bf = np.asarray(tp_weights, np.float32).reshape(NEDGES, 256).astype(NPBF)
    f_bf = np.asarray(node_feats, np.float32).astype(NPBF)
    a_bf = np.asarray(edge_attrs, np.float32).astype(NPBF)

    in_maps = [dict() for _ in range(N_CORES)]
    for c in range(N_CORES):
        for j in range(W_PER_CORE):
            w = c * W_PER_CORE + j
            e0, e1 = bounds[w], bounds[w + 1]
            n = e1 - e0
            T = int(t_slot[j])
            # edge-slot-major staging [T*128, TB] with slot-local layout:
            # [ W 0:256 | F 256:320 | A0 320:322 | A1 322:328 | A2 328:338 |
            #   A3 338:352 | S 352:480 ]
            X = np.zeros((T * 128, TB), NPBF)
            if n > 0:
                X[:n, 0:256] = w_bf[e0:e1]
                X[:n, 256:320] = f_bf[recv[e0:e1]]
                a2 = np.repeat(a_bf[e0:e1], 2, axis=1)  # [n, 32] = [lm,2] pairs
                X[:n, 320:352] = a2
                rel = (recv[e0:e1] - w * WIN).astype(np.int64)
                X[np.arange(n), 352 + rel] = NPBF(1.0)
            # Device buffer is block-major: buffer[p, blk_off + t*sz + i]
            # = X[t*128 + p, slot_off + i]
            Xt = X.reshape(T, 128, TB)
            buf = np.zeros((128, T * TB), NPBF)
            o = 0
            for so, sz in (
                (0, 256), (256, 64), (320, 2), (322, 6), (328, 10),
                (338, 14), (352, 128),
            ):
                blk = Xt[:, :, so:so + sz]  # [T, 128, sz]
                buf[:, o:o + T * sz] = blk.transpose(1, 0, 2).reshape(128, T * sz)
                o += T * sz
            in_maps[c][f"in_w{j}"] = buf
    return in_maps


def _build_program(t_slot):
    nc = bacc.Bacc("TRN2", target_bir_lowering=False, debug=False,
                   num_devices=N_CORES)
    in_d = [
        nc.dram_tensor(f"in_w{j}", [128, int(t_slot[j]) * TB], BF16,
                       kind="ExternalInput").ap()
        for j in range(W_PER_CORE)
    ]
    out_d = nc.dram_tensor("out", [W_PER_CORE * WIN, 1024], F32,
                           kind="ExternalOutput").ap()

    with tile.TileContext(nc) as tc:
        with tc.tile_pool(name="win", bufs=2) as win_pool, \
             tc.tile_pool(name="u", bufs=3) as u_pool, \
             tc.tile_pool(name="msg", bufs=3) as msg_pool, \
             tc.tile_pool(name="ps", bufs=2, space="PSUM") as ps_pool, \
             tc.tile_pool(name="ost", bufs=2) as ost_pool:
            for j in range(W_PER_CORE):
                T = int(t_slot[j])
                off = _block_offsets(T)
                win = win_pool.tile([128, T * TB], BF16, tag="win")
                nc.sync.dma_start(out=win, in_=in_d[j])
                ps = ps_pool.tile([128, 1024], F32, tag="ps")
                for b0 in range(0, T, MSG_B):
                    bn = min(MSG_B, T - b0)
                    # U = W * F  -> [128, bn, 4, 64]
                    u = u_pool.tile([128, MSG_B, 256], BF16, tag="u")
                    w_v = win[:, off["W"] + b0 * 256: off["W"] + (b0 + bn) * 256]
                    f_v = win[:, off["F"] + b0 * 64: off["F"] + (b0 + bn) * 64]
                    nc.vector.tensor_mul(
                        u[:, :bn].rearrange("p t (l c) -> p t l c", l=4),
                        w_v.rearrange("p (t l c) -> p t l c", t=bn, l=4),
                        f_v.rearrange("p (t c) -> p t c", t=bn)[
                            :, :, None, :].broadcast_to([128, bn, 4, 64]),
                    )
                    # msg[t, lm, c] = A[t, lm] * U[t, l(lm), c]
                    msg = msg_pool.tile([128, MSG_B, 16, 64], BF16, tag="msg")
                    u5 = u[:, :bn].rearrange("p t (l c d) -> p t l c d",
                                             l=4, d=2)
                    for g, (lm0, m_l) in enumerate(L_GROUPS):
                        og = off[f"A{g}"]
                        a_v = win[:, og + b0 * m_l * 2: og + (b0 + bn) * m_l * 2]
                        eng = nc.gpsimd if g <= 1 else nc.vector
                        eng.tensor_mul(
                            msg[:, :bn, lm0:lm0 + m_l, :].rearrange(
                                "p t l (c d) -> p t l c d", d=2),
                            a_v.rearrange("p (t l d) -> p t l d", t=bn, d=2)[
                                :, :, :, None, :].broadcast_to(
                                    [128, bn, m_l, 32, 2]),
                            u5[:, :, g, None, :, :].broadcast_to(
                                [128, bn, m_l, 32, 2]),
                        )
                    for b in range(bn):
                        t = b0 + b
                        s_t = win[:, off["S"] + t * 128: off["S"] + (t + 1) * 128]
                        rhs = msg[:, b].rearrange("p l c -> p (l c)")
                        nc.tensor.matmul(ps[:, 0:512], s_t, rhs[:, 0:512],
                                         start=(t == 0), stop=(t == T - 1))
                        nc.tensor.matmul(ps[:, 512:1024], s_t, rhs[:, 512:1024],
                                         start=(t == 0), stop=(t == T - 1))
                ost = ost_pool.tile([128, 1024], F32, tag="ost")
                nc.scalar.copy(ost, ps)
                nc.sync.dma_start(
                    out=out_d[j * WIN:(j + 1) * WIN, :], in_=ost)
    nc.compile()
    return nc


def kernel(node_feats, edge_attrs, tp_weights, receiver_list, nnodes,
           _trace=False):
    node_feats = np.asarray(node_feats)
    edge_attrs = np.asarray(edge_attrs)
    tp_weights = np.asarray(tp_weights)
    receiver_list = np.asarray(receiver_list)
    nnodes = int(nnodes)
    assert node_feats.shape == (NNODES, NCHAN) and nnodes == NNODES
    assert tp_weights.shape == (NEDGES, 4, NCHAN)

    bounds, t_slot = _build_schedule(receiver_list)
    key = tuple(int(x) for x in t_slot)
    if key not in _PROGRAM_CACHE:
        _PROGRAM_CACHE[key] = _build_program(t_slot)
    nc = _PROGRAM_CACHE[key]

    in_maps = _pack_inputs(node_feats, edge_attrs, tp_weights, receiver_list,
                           bounds, t_slot)
    res = run_bass_kernel_spmd(nc, in_maps, list(range(N_CORES)),
                               trace=_trace)
    pieces = [res.results[c]["out"] for c in range(N_CORES)]
    full = np.concatenate(pieces, axis=0)[:NNODES]
    out = full.reshape(NNODES, 16, NCHAN).astype(np.float32)
    if _trace:
        return out, res
    return out


# revision 4
# speedup vs baseline: 1.6242x; 1.6242x over previous
"""Trainium2 Bass kernel for InvariantMessagePassingTP.

out[n, lm, c] = sum_{e: recv[e]=n} edge_attrs[e,lm] * tp_weights[e,l(lm),c]
                * node_feats[recv[e], c]

Strategy (8 NeuronCores, SPMD, no collectives):
  receiver_list is sorted -> each core owns a contiguous node range (3125
  nodes) and its contiguous edge range. The host greedily groups nodes into
  "tiles": <=8 nodes and <=128 edges per tile. Edges sit on SBUF partitions.

  Per tile (the A-fold trick - both A and the one-hot scatter live in the
  matmul stationary):
    U[e, l*64+c]      = W[e,l,c] * F[e,c]          (DVE TT bf16 2x, batched)
    At[e, lm*8+k]     = A[e,lm]  * S8[e,k]         (DVE TT bf16 2x;
                        S8 = one-hot of the node's local index k in 0..7)
    P = At^T @ U      (PE, one matmul N=256, fp32 PSUM: P[lm*8+k, l*64+c]
                       = sum_e A*S8*W*F -- rows (lm,k), col block l(lm)
                       holds the answer)
  8 tiles share one PSUM tile; ACT then copies each l-column-block of PSUM
  (all 128 lanes) to bf16 staging, and per-l DMAs ship only the valid row
  ranges to DRAM laid out as slots[lm, k, tile, c]. The host gathers
  slots -> out[node, lm, c] (summing in the rare case a node spans tiles).
"""

import sys

sys.path.insert(0, "/opt/trn_rl_repo")

import numpy as np
import ml_dtypes

import concourse.bass as bass
import concourse.bacc as bacc
import concourse.tile as tile
from concourse import mybir
from concourse.bass_utils import run_bass_kernel_spmd

NPBF = ml_dtypes.bfloat16
BF16 = mybir.dt.bfloat16
F32 = mybir.dt.float32

NNODES = 25000
NEDGES = 400000
NCHAN = 64
N_CORES = 8
NPC = NNODES // N_CORES        # nodes per core
TB = 360                       # bf16 elems per tile per partition
CHUNK = 32                     # tiles per input DMA chunk
PSB = 8                        # tiles per PSUM batch
MSG_B = 4                      # tiles per U-op batch

L_OF_LM = np.array([0, 1, 1, 1, 2, 2, 2, 2, 2, 3, 3, 3, 3, 3, 3, 3], np.int64)
L_GROUPS = [(0, 1), (1, 3), (4, 5), (9, 7)]  # (lm_start, m_l) for l=0..3

_PROGRAM_CACHE = {}


def _greedy_groups(deg, node0):
    """Group consecutive nodes: <=8 nodes, <=128 edges per group.
    A node with deg>128 is split across several single-node groups.
    Returns list of (node_start, n_nodes, n_edges_in_group) with node-split
    groups flagged by n_nodes==1 repeats."""
    groups = []
    n = len(deg)
    i = 0
    while i < n:
        if deg[i] > 128:
            # split this node's edges over several groups
            rem = deg[i]
            while rem > 0:
                take = min(128, rem)
                groups.append((node0 + i, 1, take))
                rem -= take
            i += 1
            continue
        cnt = 0
        edges = 0
        while i + cnt < n and cnt < 8 and edges + deg[i + cnt] <= 128:
            edges += deg[i + cnt]
            cnt += 1
        groups.append((node0 + i, cnt, edges))
        i += cnt
    return groups


def _build_schedule(receiver_list):
    recv = np.asarray(receiver_list).astype(np.int64)
    deg = np.bincount(recv, minlength=NNODES)
    per_core = []
    for c in range(N_CORES):
        per_core.append(_greedy_groups(deg[c * NPC:(c + 1) * NPC], c * NPC))
    t_max = max(len(g) for g in per_core)
    t_u = -(-t_max // PSB) * PSB  # round up to PSUM batch
    return recv, deg, per_core, t_u


def _pack_inputs(node_feats, edge_attrs, tp_weights, recv, per_core, t_u):
    w_bf = np.asarray(tp_weights, np.float32).reshape(NEDGES, 256).astype(NPBF)
    f_bf = np.asarray(node_feats, np.float32).astype(NPBF)
    a_bf = np.asarray(edge_attrs, np.float32).astype(NPBF)
    # edge start index of each node (recv sorted)
    node_e0 = np.searchsorted(recv, np.arange(NNODES + 1))

    in_maps = []
    slot_maps = []  # per core: list of (node_start, n_nodes) per tile
    for c in range(N_CORES):
        groups = per_core[c]
        T = t_u
        # slot-major staging [T*128, TB]:
        # [ W 0:256 | F 256:320 | A2 320:352 | S8 352:360 ]
        X = np.zeros((T * 128, TB), NPBF)
        smap = []
        e_cursor = {}
        for t, (n0, k, ne) in enumerate(groups):
            if ne == 0:
                smap.append((n0, k))
                continue
            e0 = node_e0[n0] + e_cursor.get(n0, 0) if k == 1 else node_e0[n0]
            # for split nodes track consumed edges
            if k == 1:
                e_cursor[n0] = e_cursor.get(n0, 0) + ne
            e1 = e0 + ne
            base = t * 128
            X[base:base + ne, 0:256] = w_bf[e0:e1]
            X[base:base + ne, 256:320] = f_bf[recv[e0:e1]]
            X[base:base + ne, 320:352] = np.repeat(a_bf[e0:e1], 2, axis=1)
            loc = (recv[e0:e1] - n0).astype(np.int64)  # 0..7
            X[base + np.arange(ne), 352 + loc] = NPBF(1.0)
            smap.append((n0, k))
        while len(smap) < T:
            smap.append((0, 0))
        # chunk-block-major device layout
        Xt = X.reshape(T, 128, TB)
        n_chunks = -(-T // CHUNK)
        buf = np.zeros((128, T * TB), NPBF)
        pos = 0
        for ch in range(n_chunks):
            t0, t1 = ch * CHUNK, min((ch + 1) * CHUNK, T)
            for so, sz in ((0, 256), (256, 64), (320, 32), (352, 8)):
                blk = Xt[t0:t1, :, so:so + sz]  # [ct, 128, sz]
                ct = t1 - t0
                buf[:, pos:pos + ct * sz] = (
                    blk.transpose(1, 0, 2).reshape(128, ct * sz))
                pos += ct * sz
        in_maps.append({"inp": buf})
        slot_maps.append(smap)
    return in_maps, slot_maps


def _build_program(t_u):
    nc = bacc.Bacc("TRN2", target_bir_lowering=False, debug=False,
                   num_devices=N_CORES)
    T = t_u
    in_d = nc.dram_tensor("inp", [128, T * TB], BF16, kind="ExternalInput").ap()
    # slots[lm, k, tile, c]
    out_d = nc.dram_tensor("out", [16, 8, T, 64], BF16,
                           kind="ExternalOutput").ap()

    n_chunks = -(-T // CHUNK)
    with tile.TileContext(nc) as tc:
        with tc.tile_pool(name="ld", bufs=2) as ld_pool, \
             tc.tile_pool(name="u", bufs=3) as u_pool, \
             tc.tile_pool(name="at", bufs=3) as at_pool, \
             tc.tile_pool(name="st", bufs=2) as st_pool, \
             tc.tile_pool(name="ps", bufs=2, space="PSUM") as ps_pool:
            for ch in range(n_chunks):
                t0, t1 = ch * CHUNK, min((ch + 1) * CHUNK, T)
                ct = t1 - t0
                # chunk block offsets (bf16 elems within the chunk)
                oW, oF, oA, oS = 0, ct * 256, ct * 320, ct * 352
                base_el = t0 * TB
                ld = ld_pool.tile([128, ct * TB], BF16, tag="ld")
                nc.sync.dma_start(
                    out=ld,
                    in_=bass.AP(
                        tensor=in_d.tensor, offset=base_el,
                        ap=[[T * TB, 128], [1, ct * TB]]),
                )
                for p0 in range(0, ct, PSB):
                    ps = ps_pool.tile([128, PSB, 256], F32, tag="ps")
                    for b0 in range(p0, p0 + PSB, MSG_B):
                        bn = MSG_B
                        # U = W * F -> [128, bn, 4, 64]
                        u = u_pool.tile([128, MSG_B, 256], BF16, tag="u")
                        w_v = ld[:, oW + b0 * 256: oW + (b0 + bn) * 256]
                        f_v = ld[:, oF + b0 * 64: oF + (b0 + bn) * 64]
                        nc.vector.tensor_mul(
                            u[:, :bn].rearrange("p t (l c) -> p t l c", l=4),
                            w_v.rearrange("p (t l c) -> p t l c", t=bn, l=4),
                            f_v.rearrange("p (t c) -> p t c", t=bn)[
                                :, :, None, :].broadcast_to([128, bn, 4, 64]),
                        )
                        for b in range(bn):
                            t = b0 + b
                            # At[e, lm*8+k] = A2[e,lm,d] * S8[e,k]
                            at = at_pool.tile([128, 128], BF16, tag="at")
                            a_v = ld[:, oA + t * 32: oA + (t + 1) * 32]
                            s_v = ld[:, oS + t * 8: oS + (t + 1) * 8]
                            nc.vector.tensor_mul(
                                at.rearrange("p (l q d) -> p l q d",
                                             l=16, d=2),
                                a_v.rearrange("p (l d) -> p l d", d=2)[
                                    :, :, None, :].broadcast_to(
                                        [128, 16, 4, 2]),
                                s_v.rearrange("p (q d) -> p q d", d=2)[
                                    :, None, :, :].broadcast_to(
                                        [128, 16, 4, 2]),
                            )
                            nc.tensor.matmul(
                                ps[:, t - p0], at, u[:, b],
                                start=True, stop=True)
                    # extraction: per l, full-lane ACT copy of the column
                    # block; DMA ships only the valid (lm,k) rows.
                    for l in range(4):
                        lm0, m_l = L_GROUPS[l]
                        stg = st_pool.tile([128, PSB, 64], BF16,
                                           tag=f"stg{l}")
                        nc.scalar.copy(stg, ps[:, :, l * 64:(l + 1) * 64])
                        nc.sync.dma_start(
                            out=bass.AP(
                                tensor=out_d.tensor,
                                offset=(lm0 * 8) * (T * 64)
                                + (t0 + p0) * 64,
                                ap=[[T * 64, m_l * 8], [64, PSB], [1, 64]]),
                            in_=stg[lm0 * 8:(lm0 + m_l) * 8],
                        )
    nc.compile()
    return nc


def kernel(node_feats, edge_attrs, tp_weights, receiver_list, nnodes,
           _trace=False):
    node_feats = np.asarray(node_feats)
    edge_attrs = np.asarray(edge_attrs)
    tp_weights = np.asarray(tp_weights)
    receiver_list = np.asarray(receiver_list)
    nnodes = int(nnodes)
    assert node_feats.shape == (NNODES, NCHAN) and nnodes == NNODES
    assert tp_weights.shape == (NEDGES, 4, NCHAN)

    recv, deg, per_core, t_u = _build_schedule(receiver_list)
    key = int(t_u)
    if key not in _PROGRAM_CACHE:
        _PROGRAM_CACHE[key] = _build_program(t_u)
    nc = _PROGRAM_CACHE[key]

    in_maps, slot_maps = _pack_inputs(
        node_feats, edge_attrs, tp_weights, recv, per_core, t_u)
    res = run_bass_kernel_spmd(nc, in_maps, list(range(N_CORES)),
                               trace=_trace)

    out = np.zeros((NNODES, 16, NCHAN), np.float32)
    for c in range(N_CORES):
        slots = res.results[c]["out"].astype(np.float32)  # [16, 8, T, 64]
        smap = slot_maps[c]
        # gather/sum slots into nodes
        for t, (n0, k) in enumerate(smap):
            if k == 0:
                continue
            # slots[:, 0:k, t, :] -> out[n0:n0+k, :, :]
            out[n0:n0 + k] += slots[:, 0:k, t, :].transpose(1, 0, 2)
    if _trace:
        return out, res
    return out


# revision 6
# speedup vs baseline: 1.8598x; 1.1450x over previous
"""Trainium2 Bass kernel for InvariantMessagePassingTP.

out[n, lm, c] = sum_{e: recv[e]=n} edge_attrs[e,lm] * tp_weights[e,l(lm),c]
                * node_feats[recv[e], c]

Strategy (8 NeuronCores, SPMD, no collectives):
  receiver_list is sorted -> each core owns a contiguous node range (3125
  nodes) and its contiguous edge range. The host greedily groups nodes into
  "tiles": <=8 nodes and <=128 edges per tile. Edges sit on SBUF partitions.

  Per tile (the A-fold trick - both A and the one-hot scatter live in the
  matmul stationary):
    U[e, l*64+c]      = W[e,l,c] * F[e,c]          (DVE TT bf16 2x, batched)
    At[e, lm*8+k]     = A[e,lm]  * S8[e,k]         (DVE TT bf16 2x;
                        S8 = one-hot of the node's local index k in 0..7)
    P = At^T @ U      (PE, one matmul N=256, fp32 PSUM: P[lm*8+k, l*64+c]
                       = sum_e A*S8*W*F -- rows (lm,k), col block l(lm)
                       holds the answer)
  8 tiles share one PSUM tile; ACT then copies each l-column-block of PSUM
  (all 128 lanes) to bf16 staging, and per-l DMAs ship only the valid row
  ranges to DRAM laid out as slots[lm, k, tile, c]. The host gathers
  slots -> out[node, lm, c] (summing in the rare case a node spans tiles).
"""

import sys

sys.path.insert(0, "/opt/trn_rl_repo")

import numpy as np
import ml_dtypes

import concourse.bass as bass
import concourse.bacc as bacc
import concourse.tile as tile
from concourse import mybir
from concourse.bass_utils import run_bass_kernel_spmd

NPBF = ml_dtypes.bfloat16
BF16 = mybir.dt.bfloat16
F32 = mybir.dt.float32

NNODES = 25000
NEDGES = 400000
NCHAN = 64
N_CORES = 8
NPC = NNODES // N_CORES        # nodes per core
TB = 360                       # bf16 elems per tile per partition
CHUNK = 32                     # tiles per input DMA chunk
PSB = 8                        # tiles per PSUM batch
MSG_B = 4                      # tiles per U-op batch

L_OF_LM = np.array([0, 1, 1, 1, 2, 2, 2, 2, 2, 3, 3, 3, 3, 3, 3, 3], np.int64)
L_GROUPS = [(0, 1), (1, 3), (4, 5), (9, 7)]  # (lm_start, m_l) for l=0..3
# row-block order of lm in At / PSUM / slots: l2,l3 first (96 rows at psum
# base 0), then l0,l1 (32 rows at base 96) - matmul psum-base constraint.
PERM_LM = [4, 5, 6, 7, 8, 9, 10, 11, 12, 13, 14, 15, 0, 1, 2, 3]

_PROGRAM_CACHE = {}


def _greedy_groups(deg, node0):
    """Group consecutive nodes: <=8 nodes, <=128 edges per group.
    A node with deg>128 is split across several single-node groups.
    Returns list of (node_start, n_nodes, n_edges_in_group) with node-split
    groups flagged by n_nodes==1 repeats."""
    groups = []
    n = len(deg)
    i = 0
    while i < n:
        if deg[i] > 128:
            # split this node's edges over several groups
            rem = deg[i]
            while rem > 0:
                take = min(128, rem)
                groups.append((node0 + i, 1, take))
                rem -= take
            i += 1
            continue
        cnt = 0
        edges = 0
        while i + cnt < n and cnt < 8 and edges + deg[i + cnt] <= 128:
            edges += deg[i + cnt]
            cnt += 1
        groups.append((node0 + i, cnt, edges))
        i += cnt
    return groups


def _build_schedule(receiver_list):
    recv = np.asarray(receiver_list).astype(np.int64)
    deg = np.bincount(recv, minlength=NNODES)
    per_core = []
    for c in range(N_CORES):
        per_core.append(_greedy_groups(deg[c * NPC:(c + 1) * NPC], c * NPC))
    t_max = max(len(g) for g in per_core)
    t_u = -(-t_max // PSB) * PSB  # round up to PSUM batch
    return recv, deg, per_core, t_u


def _pack_inputs(node_feats, edge_attrs, tp_weights, recv, per_core, t_u):
    w_bf = np.asarray(tp_weights, np.float32).reshape(NEDGES, 256).astype(NPBF)
    f_bf = np.asarray(node_feats, np.float32).astype(NPBF)
    a_bf = np.asarray(edge_attrs, np.float32).astype(NPBF)
    # edge start index of each node (recv sorted)
    node_e0 = np.searchsorted(recv, np.arange(NNODES + 1))

    in_maps = []
    slot_maps = []  # per core: list of (node_start, n_nodes) per tile
    for c in range(N_CORES):
        groups = per_core[c]
        T = t_u
        # slot-major staging [T*128, TB]:
        # [ W 0:256 | F 256:320 | A2 320:352 | S8 352:360 ]
        X = np.zeros((T * 128, TB), NPBF)
        smap = []
        e_cursor = {}
        for t, (n0, k, ne) in enumerate(groups):
            if ne == 0:
                smap.append((n0, k))
                continue
            e0 = node_e0[n0] + e_cursor.get(n0, 0) if k == 1 else node_e0[n0]
            # for split nodes track consumed edges
            if k == 1:
                e_cursor[n0] = e_cursor.get(n0, 0) + ne
            e1 = e0 + ne
            base = t * 128
            X[base:base + ne, 0:256] = w_bf[e0:e1]
            X[base:base + ne, 256:320] = f_bf[recv[e0:e1]]
            a2 = np.repeat(a_bf[e0:e1][:, PERM_LM], 2, axis=1)
            X[base:base + ne, 320:352] = a2
            loc = (recv[e0:e1] - n0).astype(np.int64)  # 0..7
            X[base + np.arange(ne), 352 + loc] = NPBF(1.0)
            smap.append((n0, k))
        while len(smap) < T:
            smap.append((0, 0))
        # chunk-block-major device layout
        Xt = X.reshape(T, 128, TB)
        n_chunks = -(-T // CHUNK)
        buf = np.zeros((128, T * TB), NPBF)
        pos = 0
        for ch in range(n_chunks):
            t0, t1 = ch * CHUNK, min((ch + 1) * CHUNK, T)
            for so, sz in ((0, 256), (256, 64), (320, 32), (352, 8)):
                blk = Xt[t0:t1, :, so:so + sz]  # [ct, 128, sz]
                ct = t1 - t0
                buf[:, pos:pos + ct * sz] = (
                    blk.transpose(1, 0, 2).reshape(128, ct * sz))
                pos += ct * sz
        in_maps.append({"inp": buf})
        slot_maps.append(smap)
    return in_maps, slot_maps


def _build_program(t_u):
    nc = bacc.Bacc("TRN2", target_bir_lowering=False, debug=False,
                   num_devices=N_CORES)
    T = t_u
    in_d = nc.dram_tensor("inp", [128, T * TB], BF16, kind="ExternalInput").ap()
    # slots[row = perm-lm-block*8 + k, tile, c]
    out_d = nc.dram_tensor("out", [128, T, 64], BF16,
                           kind="ExternalOutput").ap()

    n_chunks = -(-T // CHUNK)
    with tile.TileContext(nc) as tc:
        with tc.tile_pool(name="ld", bufs=2) as ld_pool, \
             tc.tile_pool(name="u", bufs=3) as u_pool, \
             tc.tile_pool(name="at", bufs=3) as at_pool, \
             tc.tile_pool(name="st", bufs=2) as st_pool, \
             tc.tile_pool(name="ps", bufs=2, space="PSUM") as ps_pool:
            for ch in range(n_chunks):
                t0, t1 = ch * CHUNK, min((ch + 1) * CHUNK, T)
                ct = t1 - t0
                # chunk block offsets (bf16 elems within the chunk)
                oW, oF, oA, oS = 0, ct * 256, ct * 320, ct * 352
                base_el = t0 * TB
                ld = ld_pool.tile([128, ct * TB], BF16, tag="ld")
                nc.sync.dma_start(
                    out=ld,
                    in_=bass.AP(
                        tensor=in_d.tensor, offset=base_el,
                        ap=[[T * TB, 128], [1, ct * TB]]),
                )
                # per-chunk staging: [128, half, ct, 64] bf16
                stage = st_pool.tile([128, 2, ct, 64], BF16, tag="stage")
                for p0 in range(0, ct, PSB):
                    ps = ps_pool.tile([128, PSB, 128], F32, tag="ps")
                    for b0 in range(p0, p0 + PSB, MSG_B):
                        bn = MSG_B
                        # U = W * F -> [128, bn, 4, 64]
                        u = u_pool.tile([128, MSG_B, 256], BF16, tag="u")
                        w_v = ld[:, oW + b0 * 256: oW + (b0 + bn) * 256]
                        f_v = ld[:, oF + b0 * 64: oF + (b0 + bn) * 64]
                        nc.vector.tensor_mul(
                            u[:, :bn].rearrange("p t (l c) -> p t l c", l=4),
                            w_v.rearrange("p (t l c) -> p t l c", t=bn, l=4),
                            f_v.rearrange("p (t c) -> p t c", t=bn)[
                                :, :, None, :].broadcast_to([128, bn, 4, 64]),
                        )
                        for b in range(bn):
                            t = b0 + b
                            # At[e, lm*8+k] = A2[e,lm,d] * S8[e,k]
                            at = at_pool.tile([128, 128], BF16, tag="at")
                            a_v = ld[:, oA + t * 32: oA + (t + 1) * 32]
                            s_v = ld[:, oS + t * 8: oS + (t + 1) * 8]
                            nc.vector.tensor_mul(
                                at.rearrange("p (l q d) -> p l q d",
                                             l=16, d=2),
                                a_v.rearrange("p (l d) -> p l d", d=2)[
                                    :, :, None, :].broadcast_to(
                                        [128, 16, 4, 2]),
                                s_v.rearrange("p (q d) -> p q d", d=2)[
                                    :, None, :, :].broadcast_to(
                                        [128, 16, 4, 2]),
                            )
                            # rows 0-95 = (l2|l3) x U cols 128:256,
                            # rows 96-127 = (l0|l1) x U cols 0:128
                            nc.tensor.matmul(
                                ps[0:96, t - p0], at[:, 0:96],
                                u[:, b, 128:256], start=True, stop=True)
                            nc.tensor.matmul(
                                ps[96:128, t - p0], at[:, 96:128],
                                u[:, b, 0:128], start=True, stop=True,
                                tile_position=(0, 96))
                    # full-lane extraction of the whole PSUM batch into the
                    # chunk stage, col halves separated for contiguous DMA
                    nc.scalar.copy(
                        bass.AP(
                            tensor=stage.tensor, offset=stage.offset + p0 * 64,
                            ap=[stage.ap[0], [64, PSB], [ct * 64, 2],
                                [1, 64]]),
                        ps,
                    )
                # 4 out-DMA fragments per chunk; DMA picks valid rows
                for (r0, r1, half) in ((0, 40, 0), (40, 96, 1),
                                       (96, 104, 0), (104, 128, 1)):
                    nc.sync.dma_start(
                        out=bass.AP(
                            tensor=out_d.tensor,
                            offset=r0 * (T * 64) + t0 * 64,
                            ap=[[T * 64, r1 - r0], [64, ct], [1, 64]]),
                        in_=stage[r0:r1, half],
                    )
    nc.compile()
    return nc


def kernel(node_feats, edge_attrs, tp_weights, receiver_list, nnodes,
           _trace=False):
    node_feats = np.asarray(node_feats)
    edge_attrs = np.asarray(edge_attrs)
    tp_weights = np.asarray(tp_weights)
    receiver_list = np.asarray(receiver_list)
    nnodes = int(nnodes)
    assert node_feats.shape == (NNODES, NCHAN) and nnodes == NNODES
    assert tp_weights.shape == (NEDGES, 4, NCHAN)

    recv, deg, per_core, t_u = _build_schedule(receiver_list)
    key = int(t_u)
    if key not in _PROGRAM_CACHE:
        _PROGRAM_CACHE[key] = _build_program(t_u)
    nc = _PROGRAM_CACHE[key]

    in_maps, slot_maps = _pack_inputs(
        node_feats, edge_attrs, tp_weights, recv, per_core, t_u)
    res = run_bass_kernel_spmd(nc, in_maps, list(range(N_CORES)),
                               trace=_trace)

    inv = np.argsort(np.array(PERM_LM))  # lm -> row-block index
    out = np.zeros((NNODES, 16, NCHAN), np.float32)
    for c in range(N_CORES):
        slots = res.results[c]["out"].astype(np.float32)  # [128, T, 64]
        slots = slots.reshape(16, 8, -1, NCHAN)[inv]  # [lm, k, T, c]
        smap = slot_maps[c]
        for t, (n0, k) in enumerate(smap):
            if k == 0:
                continue
            out[n0:n0 + k] += slots[:, 0:k, t, :].transpose(1, 0, 2)
    if _trace:
        return out, res
    return out


# revision 7
# speedup vs baseline: 2.1029x; 1.1307x over previous
"""Trainium2 Bass kernel for InvariantMessagePassingTP.

out[n, lm, c] = sum_{e: recv[e]=n} edge_attrs[e,lm] * tp_weights[e,l(lm),c]
                * node_feats[recv[e], c]

Strategy (8 NeuronCores, SPMD, no collectives):
  receiver_list is sorted -> each core owns a contiguous node range (3125
  nodes) and its contiguous edge range. The host greedily groups nodes into
  "tiles": <=8 nodes and <=128 edges per tile. Edges sit on SBUF partitions.

  Per tile (the A-fold trick - both A and the one-hot scatter live in the
  matmul stationary):
    U[e, l*64+c]      = W[e,l,c] * F[e,c]          (DVE TT bf16 2x, batched)
    At[e, lm*8+k]     = A[e,lm]  * S8[e,k]         (DVE TT bf16 2x;
                        S8 = one-hot of the node's local index k in 0..7)
    P = At^T @ U      (PE, one matmul N=256, fp32 PSUM: P[lm*8+k, l*64+c]
                       = sum_e A*S8*W*F -- rows (lm,k), col block l(lm)
                       holds the answer)
  8 tiles share one PSUM tile; ACT then copies each l-column-block of PSUM
  (all 128 lanes) to bf16 staging, and per-l DMAs ship only the valid row
  ranges to DRAM laid out as slots[lm, k, tile, c]. The host gathers
  slots -> out[node, lm, c] (summing in the rare case a node spans tiles).
"""

import sys

sys.path.insert(0, "/opt/trn_rl_repo")

import numpy as np
import ml_dtypes

import concourse.bass as bass
import concourse.bacc as bacc
import concourse.tile as tile
from concourse import mybir
from concourse.bass_utils import run_bass_kernel_spmd

NPBF = ml_dtypes.bfloat16
BF16 = mybir.dt.bfloat16
F32 = mybir.dt.float32

NNODES = 25000
NEDGES = 400000
NCHAN = 64
N_CORES = 8
NPC = NNODES // N_CORES        # nodes per core
TB = 360                       # bf16 elems per tile per partition
CHUNK = 32                     # tiles per input DMA chunk
PSB = 8                        # tiles per PSUM batch
MSG_B = 4                      # tiles per U-op batch

L_OF_LM = np.array([0, 1, 1, 1, 2, 2, 2, 2, 2, 3, 3, 3, 3, 3, 3, 3], np.int64)
L_GROUPS = [(0, 1), (1, 3), (4, 5), (9, 7)]  # (lm_start, m_l) for l=0..3
# row-block order of lm in At / PSUM / slots: l2,l3 first (96 rows at psum
# base 0), then l0,l1 (32 rows at base 96) - matmul psum-base constraint.
PERM_LM = [4, 5, 6, 7, 8, 9, 10, 11, 12, 13, 14, 15, 0, 1, 2, 3]

_PROGRAM_CACHE = {}


def _greedy_groups(deg, node0):
    """Group consecutive nodes: <=8 nodes, <=128 edges per group.
    A node with deg>128 is split across several single-node groups.
    Returns list of (node_start, n_nodes, n_edges_in_group) with node-split
    groups flagged by n_nodes==1 repeats."""
    groups = []
    n = len(deg)
    i = 0
    while i < n:
        if deg[i] > 128:
            # split this node's edges over several groups
            rem = deg[i]
            while rem > 0:
                take = min(128, rem)
                groups.append((node0 + i, 1, take))
                rem -= take
            i += 1
            continue
        cnt = 0
        edges = 0
        while i + cnt < n and cnt < 8 and edges + deg[i + cnt] <= 128:
            edges += deg[i + cnt]
            cnt += 1
        groups.append((node0 + i, cnt, edges))
        i += cnt
    return groups


def _build_schedule(receiver_list):
    recv = np.asarray(receiver_list).astype(np.int64)
    deg = np.bincount(recv, minlength=NNODES)
    per_core = []
    for c in range(N_CORES):
        per_core.append(_greedy_groups(deg[c * NPC:(c + 1) * NPC], c * NPC))
    t_max = max(len(g) for g in per_core)
    t_u = -(-t_max // PSB) * PSB  # round up to PSUM batch
    return recv, deg, per_core, t_u


def _pack_inputs(node_feats, edge_attrs, tp_weights, recv, per_core, t_u):
    w_bf = np.asarray(tp_weights, np.float32).reshape(NEDGES, 256).astype(NPBF)
    f_bf = np.asarray(node_feats, np.float32).astype(NPBF)
    a_bf = np.asarray(edge_attrs, np.float32).astype(NPBF)
    # edge start index of each node (recv sorted)
    node_e0 = np.searchsorted(recv, np.arange(NNODES + 1))

    in_maps = []
    slot_maps = []  # per core: list of (node_start, n_nodes) per tile
    for c in range(N_CORES):
        groups = per_core[c]
        T = t_u
        # slot-major staging [T*128, TB]:
        # [ W 0:256 | F 256:320 | A2 320:352 | S8 352:360 ]
        X = np.zeros((T * 128, TB), NPBF)
        smap = []
        e_cursor = {}
        for t, (n0, k, ne) in enumerate(groups):
            if ne == 0:
                smap.append((n0, k))
                continue
            e0 = node_e0[n0] + e_cursor.get(n0, 0) if k == 1 else node_e0[n0]
            # for split nodes track consumed edges
            if k == 1:
                e_cursor[n0] = e_cursor.get(n0, 0) + ne
            e1 = e0 + ne
            base = t * 128
            X[base:base + ne, 0:256] = w_bf[e0:e1]
            X[base:base + ne, 256:320] = f_bf[recv[e0:e1]]
            a2 = np.repeat(a_bf[e0:e1][:, PERM_LM], 2, axis=1)
            X[base:base + ne, 320:352] = a2
            loc = (recv[e0:e1] - n0).astype(np.int64)  # 0..7
            X[base + np.arange(ne), 352 + loc] = NPBF(1.0)
            smap.append((n0, k))
        while len(smap) < T:
            smap.append((0, 0))
        # chunk-block-major device layout
        Xt = X.reshape(T, 128, TB)
        n_chunks = -(-T // CHUNK)
        buf = np.zeros((128, T * TB), NPBF)
        pos = 0
        for ch in range(n_chunks):
            t0, t1 = ch * CHUNK, min((ch + 1) * CHUNK, T)
            for so, sz in ((0, 256), (256, 64), (320, 32), (352, 8)):
                blk = Xt[t0:t1, :, so:so + sz]  # [ct, 128, sz]
                ct = t1 - t0
                buf[:, pos:pos + ct * sz] = (
                    blk.transpose(1, 0, 2).reshape(128, ct * sz))
                pos += ct * sz
        in_maps.append({"inp": buf})
        slot_maps.append(smap)
    return in_maps, slot_maps


def _build_program(t_u):
    nc = bacc.Bacc("TRN2", target_bir_lowering=False, debug=False,
                   num_devices=N_CORES)
    T = t_u
    in_d = nc.dram_tensor("inp", [128, T * TB], BF16, kind="ExternalInput").ap()
    # slots[row = perm-lm-block*8 + k, tile, c]
    out_d = nc.dram_tensor("out", [128, T, 64], BF16,
                           kind="ExternalOutput").ap()

    n_chunks = -(-T // CHUNK)
    with tile.TileContext(nc) as tc:
        with tc.tile_pool(name="ld", bufs=2) as ld_pool, \
             tc.tile_pool(name="u", bufs=4) as u_pool, \
             tc.tile_pool(name="at", bufs=12) as at_pool, \
             tc.tile_pool(name="st", bufs=2) as st_pool, \
             tc.tile_pool(name="ps", bufs=2, space="PSUM") as ps_pool:
            for ch in range(n_chunks):
                t0, t1 = ch * CHUNK, min((ch + 1) * CHUNK, T)
                ct = t1 - t0
                # chunk block offsets (bf16 elems within the chunk)
                oW, oF, oA, oS = 0, ct * 256, ct * 320, ct * 352
                base_el = t0 * TB
                ld = ld_pool.tile([128, ct * TB], BF16, tag="ld")
                nc.sync.dma_start(
                    out=ld,
                    in_=bass.AP(
                        tensor=in_d.tensor, offset=base_el,
                        ap=[[T * TB, 128], [1, ct * TB]]),
                )
                # per-chunk staging: [128, half, ct, 64] bf16
                stage = st_pool.tile([128, 2, ct, 64], BF16, tag="stage")
                for p0 in range(0, ct, PSB):
                    ps = ps_pool.tile([128, PSB, 128], F32, tag="ps")
                    ats = []
                    us = []
                    for b0 in range(p0, p0 + PSB, MSG_B):
                        bn = MSG_B
                        # U = W * F -> [128, bn, 4, 64]
                        u = u_pool.tile([128, MSG_B, 256], BF16, tag="u")
                        us.append(u)
                        w_v = ld[:, oW + b0 * 256: oW + (b0 + bn) * 256]
                        f_v = ld[:, oF + b0 * 64: oF + (b0 + bn) * 64]
                        nc.vector.tensor_mul(
                            u[:, :bn].rearrange("p t (l c) -> p t l c", l=4),
                            w_v.rearrange("p (t l c) -> p t l c", t=bn, l=4),
                            f_v.rearrange("p (t c) -> p t c", t=bn)[
                                :, :, None, :].broadcast_to([128, bn, 4, 64]),
                        )
                        for b in range(bn):
                            t = b0 + b
                            # At[e, lm*8+k] = A2[e,lm,d] * S8[e,k]
                            at = at_pool.tile([128, 128], BF16, tag="at")
                            ats.append(at)
                            a_v = ld[:, oA + t * 32: oA + (t + 1) * 32]
                            s_v = ld[:, oS + t * 8: oS + (t + 1) * 8]
                            nc.vector.tensor_mul(
                                at.rearrange("p (l q d) -> p l q d",
                                             l=16, d=2),
                                a_v.rearrange("p (l d) -> p l d", d=2)[
                                    :, :, None, :].broadcast_to(
                                        [128, 16, 4, 2]),
                                s_v.rearrange("p (q d) -> p q d", d=2)[
                                    :, None, :, :].broadcast_to(
                                        [128, 16, 4, 2]),
                            )
                    # phase A: rows 0-95 = (l2|l3) x U cols 128:256
                    for k in range(PSB):
                        nc.tensor.matmul(
                            ps[0:96, k], ats[k][:, 0:96],
                            us[k // MSG_B][:, k % MSG_B, 128:256],
                            start=True, stop=True)
                    # phase B: rows 96-127 = (l0|l1) x U cols 0:128
                    for k in range(PSB):
                        nc.tensor.matmul(
                            ps[96:128, k], ats[k][:, 96:128],
                            us[k // MSG_B][:, k % MSG_B, 0:128],
                            start=True, stop=True,
                            tile_position=(0, 96))
                    # full-lane extraction of the whole PSUM batch into the
                    # chunk stage, col halves separated for contiguous DMA
                    nc.scalar.copy(
                        bass.AP(
                            tensor=stage.tensor, offset=stage.offset + p0 * 64,
                            ap=[stage.ap[0], [64, PSB], [ct * 64, 2],
                                [1, 64]]),
                        ps,
                    )
                # 4 out-DMA fragments per chunk; DMA picks valid rows
                for (r0, r1, half) in ((0, 40, 0), (40, 96, 1),
                                       (96, 104, 0), (104, 128, 1)):
                    nc.sync.dma_start(
                        out=bass.AP(
                            tensor=out_d.tensor,
                            offset=r0 * (T * 64) + t0 * 64,
                            ap=[[T * 64, r1 - r0], [64, ct], [1, 64]]),
                        in_=stage[r0:r1, half],
                    )
    nc.compile()
    return nc


def kernel(node_feats, edge_attrs, tp_weights, receiver_list, nnodes,
           _trace=False):
    node_feats = np.asarray(node_feats)
    edge_attrs = np.asarray(edge_attrs)
    tp_weights = np.asarray(tp_weights)
    receiver_list = np.asarray(receiver_list)
    nnodes = int(nnodes)
    assert node_feats.shape == (NNODES, NCHAN) and nnodes == NNODES
    assert tp_weights.shape == (NEDGES, 4, NCHAN)

    recv, deg, per_core, t_u = _build_schedule(receiver_list)
    key = int(t_u)
    if key not in _PROGRAM_CACHE:
        _PROGRAM_CACHE[key] = _build_program(t_u)
    nc = _PROGRAM_CACHE[key]

    in_maps, slot_maps = _pack_inputs(
        node_feats, edge_attrs, tp_weights, recv, per_core, t_u)
    res = run_bass_kernel_spmd(nc, in_maps, list(range(N_CORES)),
                               trace=_trace)

    inv = np.argsort(np.array(PERM_LM))  # lm -> row-block index
    out = np.zeros((NNODES, 16, NCHAN), np.float32)
    for c in range(N_CORES):
        slots = res.results[c]["out"].astype(np.float32)  # [128, T, 64]
        slots = slots.reshape(16, 8, -1, NCHAN)[inv]  # [lm, k, T, c]
        smap = slot_maps[c]
        for t, (n0, k) in enumerate(smap):
            if k == 0:
                continue
            out[n0:n0 + k] += slots[:, 0:k, t, :].transpose(1, 0, 2)
    if _trace:
        return out, res
    return out


# revision 8
# speedup vs baseline: 2.1205x; 1.0084x over previous
"""Trainium2 Bass kernel for InvariantMessagePassingTP.

out[n, lm, c] = sum_{e: recv[e]=n} edge_attrs[e,lm] * tp_weights[e,l(lm),c]
                * node_feats[recv[e], c]

Strategy (8 NeuronCores, SPMD, no collectives):
  receiver_list is sorted -> each core owns a contiguous node range (3125
  nodes) and its contiguous edge range. The host greedily groups nodes into
  "tiles": <=8 nodes and <=128 edges per tile. Edges sit on SBUF partitions.

  Per tile (the A-fold trick - both A and the one-hot scatter live in the
  matmul stationary):
    U[e, l*64+c]      = W[e,l,c] * F[e,c]          (DVE TT bf16 2x, batched)
    At[e, lm*8+k]     = A[e,lm]  * S8[e,k]         (DVE TT bf16 2x;
                        S8 = one-hot of the node's local index k in 0..7)
    P = At^T @ U      (PE, one matmul N=256, fp32 PSUM: P[lm*8+k, l*64+c]
                       = sum_e A*S8*W*F -- rows (lm,k), col block l(lm)
                       holds the answer)
  8 tiles share one PSUM tile; ACT then copies each l-column-block of PSUM
  (all 128 lanes) to bf16 staging, and per-l DMAs ship only the valid row
  ranges to DRAM laid out as slots[lm, k, tile, c]. The host gathers
  slots -> out[node, lm, c] (summing in the rare case a node spans tiles).
"""

import sys

sys.path.insert(0, "/opt/trn_rl_repo")

import numpy as np
import ml_dtypes

import concourse.bass as bass
import concourse.bacc as bacc
import concourse.tile as tile
from concourse import mybir
from concourse.bass_utils import run_bass_kernel_spmd

NPBF = ml_dtypes.bfloat16
BF16 = mybir.dt.bfloat16
F32 = mybir.dt.float32

NNODES = 25000
NEDGES = 400000
NCHAN = 64
N_CORES = 8
NPC = NNODES // N_CORES        # nodes per core
TB = 360                       # bf16 elems per tile per partition
CHUNK = 32                     # tiles per input DMA chunk
PSB = 8                        # tiles per PSUM batch
MSG_B = 4                      # tiles per U-op batch

L_OF_LM = np.array([0, 1, 1, 1, 2, 2, 2, 2, 2, 3, 3, 3, 3, 3, 3, 3], np.int64)
L_GROUPS = [(0, 1), (1, 3), (4, 5), (9, 7)]  # (lm_start, m_l) for l=0..3
# row-block order of lm in At / PSUM / slots: l2,l3 first (96 rows at psum
# base 0), then l0,l1 (32 rows at base 96) - matmul psum-base constraint.
PERM_LM = [4, 5, 6, 7, 8, 9, 10, 11, 12, 13, 14, 15, 0, 1, 2, 3]

_PROGRAM_CACHE = {}


def _greedy_groups(deg, node0):
    """Group consecutive nodes: <=8 nodes, <=128 edges per group.
    A node with deg>128 is split across several single-node groups.
    Returns list of (node_start, n_nodes, n_edges_in_group) with node-split
    groups flagged by n_nodes==1 repeats."""
    groups = []
    n = len(deg)
    i = 0
    while i < n:
        if deg[i] > 128:
            # split this node's edges over several groups
            rem = deg[i]
            while rem > 0:
                take = min(128, rem)
                groups.append((node0 + i, 1, take))
                rem -= take
            i += 1
            continue
        cnt = 0
        edges = 0
        while i + cnt < n and cnt < 8 and edges + deg[i + cnt] <= 128:
            edges += deg[i + cnt]
            cnt += 1
        groups.append((node0 + i, cnt, edges))
        i += cnt
    return groups


def _build_schedule(receiver_list):
    recv = np.asarray(receiver_list).astype(np.int64)
    deg = np.bincount(recv, minlength=NNODES)
    per_core = []
    for c in range(N_CORES):
        per_core.append(_greedy_groups(deg[c * NPC:(c + 1) * NPC], c * NPC))
    t_max = max(len(g) for g in per_core)
    t_u = -(-t_max // PSB) * PSB  # round up to PSUM batch
    return recv, deg, per_core, t_u


def _pack_inputs(node_feats, edge_attrs, tp_weights, recv, per_core, t_u):
    w_bf = np.asarray(tp_weights, np.float32).reshape(NEDGES, 256).astype(NPBF)
    f_bf = np.asarray(node_feats, np.float32).astype(NPBF)
    a_bf = np.asarray(edge_attrs, np.float32).astype(NPBF)
    # edge start index of each node (recv sorted)
    node_e0 = np.searchsorted(recv, np.arange(NNODES + 1))

    in_maps = []
    slot_maps = []  # per core: list of (node_start, n_nodes) per tile
    for c in range(N_CORES):
        groups = per_core[c]
        T = t_u
        # slot-major staging [T*128, TB]:
        # [ W 0:256 | F 256:320 | A2 320:352 | S8 352:360 ]
        X = np.zeros((T * 128, TB), NPBF)
        smap = []
        e_cursor = {}
        for t, (n0, k, ne) in enumerate(groups):
            if ne == 0:
                smap.append((n0, k))
                continue
            e0 = node_e0[n0] + e_cursor.get(n0, 0) if k == 1 else node_e0[n0]
            # for split nodes track consumed edges
            if k == 1:
                e_cursor[n0] = e_cursor.get(n0, 0) + ne
            e1 = e0 + ne
            base = t * 128
            X[base:base + ne, 0:256] = w_bf[e0:e1]
            X[base:base + ne, 256:320] = f_bf[recv[e0:e1]]
            a2 = np.repeat(a_bf[e0:e1][:, PERM_LM], 2, axis=1)
            X[base:base + ne, 320:352] = a2
            loc = (recv[e0:e1] - n0).astype(np.int64)  # 0..7
            X[base + np.arange(ne), 352 + loc] = NPBF(1.0)
            smap.append((n0, k))
        while len(smap) < T:
            smap.append((0, 0))
        # chunk-block-major device layout
        Xt = X.reshape(T, 128, TB)
        n_chunks = -(-T // CHUNK)
        buf = np.zeros((128, T * TB), NPBF)
        pos = 0
        for ch in range(n_chunks):
            t0, t1 = ch * CHUNK, min((ch + 1) * CHUNK, T)
            for so, sz in ((0, 256), (256, 64), (320, 32), (352, 8)):
                blk = Xt[t0:t1, :, so:so + sz]  # [ct, 128, sz]
                ct = t1 - t0
                buf[:, pos:pos + ct * sz] = (
                    blk.transpose(1, 0, 2).reshape(128, ct * sz))
                pos += ct * sz
        in_maps.append({"inp": buf})
        slot_maps.append(smap)
    return in_maps, slot_maps


def _build_program(t_u):
    nc = bacc.Bacc("TRN2", target_bir_lowering=False, debug=False,
                   num_devices=N_CORES)
    T = t_u
    in_d = nc.dram_tensor("inp", [128, T * TB], BF16, kind="ExternalInput").ap()
    # slots[row = perm-lm-block*8 + k, tile, c]
    out_d = nc.dram_tensor("out", [128, T, 64], BF16,
                           kind="ExternalOutput").ap()

    n_chunks = -(-T // CHUNK)
    with tile.TileContext(nc) as tc:
        with tc.tile_pool(name="ld", bufs=3) as ld_pool, \
             tc.tile_pool(name="u", bufs=6) as u_pool, \
             tc.tile_pool(name="at", bufs=20) as at_pool, \
             tc.tile_pool(name="st", bufs=3) as st_pool, \
             tc.tile_pool(name="ps", bufs=4, space="PSUM") as ps_pool:
            for ch in range(n_chunks):
                t0, t1 = ch * CHUNK, min((ch + 1) * CHUNK, T)
                ct = t1 - t0
                # chunk block offsets (bf16 elems within the chunk)
                oW, oF, oA, oS = 0, ct * 256, ct * 320, ct * 352
                base_el = t0 * TB
                ld = ld_pool.tile([128, ct * TB], BF16, tag="ld")
                nc.sync.dma_start(
                    out=ld,
                    in_=bass.AP(
                        tensor=in_d.tensor, offset=base_el,
                        ap=[[T * TB, 128], [1, ct * TB]]),
                )
                # per-chunk staging: [128, half, ct, 64] bf16
                stage = st_pool.tile([128, 2, ct, 64], BF16, tag="stage")
                for p0 in range(0, ct, PSB):
                    ps = ps_pool.tile([128, PSB, 128], F32, tag="ps")
                    ats = []
                    us = []
                    for b0 in range(p0, p0 + PSB, MSG_B):
                        bn = MSG_B
                        # U = W * F -> [128, bn, 4, 64]
                        u = u_pool.tile([128, MSG_B, 256], BF16, tag="u")
                        us.append(u)
                        w_v = ld[:, oW + b0 * 256: oW + (b0 + bn) * 256]
                        f_v = ld[:, oF + b0 * 64: oF + (b0 + bn) * 64]
                        nc.vector.tensor_mul(
                            u[:, :bn].rearrange("p t (l c) -> p t l c", l=4),
                            w_v.rearrange("p (t l c) -> p t l c", t=bn, l=4),
                            f_v.rearrange("p (t c) -> p t c", t=bn)[
                                :, :, None, :].broadcast_to([128, bn, 4, 64]),
                        )
                        for b in range(bn):
                            t = b0 + b
                            # At[e, lm*8+k] = A2[e,lm,d] * S8[e,k]
                            at = at_pool.tile([128, 128], BF16, tag="at")
                            ats.append(at)
                            a_v = ld[:, oA + t * 32: oA + (t + 1) * 32]
                            s_v = ld[:, oS + t * 8: oS + (t + 1) * 8]
                            nc.vector.tensor_mul(
                                at.rearrange("p (l q d) -> p l q d",
                                             l=16, d=2),
                                a_v.rearrange("p (l d) -> p l d", d=2)[
                                    :, :, None, :].broadcast_to(
                                        [128, 16, 4, 2]),
                                s_v.rearrange("p (q d) -> p q d", d=2)[
                                    :, None, :, :].broadcast_to(
                                        [128, 16, 4, 2]),
                            )
                    # phase A: rows 0-95 = (l2|l3) x U cols 128:256
                    for k in range(PSB):
                        nc.tensor.matmul(
                            ps[0:96, k], ats[k][:, 0:96],
                            us[k // MSG_B][:, k % MSG_B, 128:256],
                            start=True, stop=True)
                    # phase B: rows 96-127 = (l0|l1) x U cols 0:128
                    for k in range(PSB):
                        nc.tensor.matmul(
                            ps[96:128, k], ats[k][:, 96:128],
                            us[k // MSG_B][:, k % MSG_B, 0:128],
                            start=True, stop=True,
                            tile_position=(0, 96))
                    # full-lane extraction of the whole PSUM batch into the
                    # chunk stage, col halves separated for contiguous DMA
                    nc.scalar.copy(
                        bass.AP(
                            tensor=stage.tensor, offset=stage.offset + p0 * 64,
                            ap=[stage.ap[0], [64, PSB], [ct * 64, 2],
                                [1, 64]]),
                        ps,
                    )
                # 4 out-DMA fragments per chunk; DMA picks valid rows
                for (r0, r1, half) in ((0, 40, 0), (40, 96, 1),
                                       (96, 104, 0), (104, 128, 1)):
                    nc.sync.dma_start(
                        out=bass.AP(
                            tensor=out_d.tensor,
                            offset=r0 * (T * 64) + t0 * 64,
                            ap=[[T * 64, r1 - r0], [64, ct], [1, 64]]),
                        in_=stage[r0:r1, half],
                    )
    nc.compile()
    return nc


def kernel(node_feats, edge_attrs, tp_weights, receiver_list, nnodes,
           _trace=False):
    node_feats = np.asarray(node_feats)
    edge_attrs = np.asarray(edge_attrs)
    tp_weights = np.asarray(tp_weights)
    receiver_list = np.asarray(receiver_list)
    nnodes = int(nnodes)
    assert node_feats.shape == (NNODES, NCHAN) and nnodes == NNODES
    assert tp_weights.shape == (NEDGES, 4, NCHAN)

    recv, deg, per_core, t_u = _build_schedule(receiver_list)
    key = int(t_u)
    if key not in _PROGRAM_CACHE:
        _PROGRAM_CACHE[key] = _build_program(t_u)
    nc = _PROGRAM_CACHE[key]

    in_maps, slot_maps = _pack_inputs(
        node_feats, edge_attrs, tp_weights, recv, per_core, t_u)
    res = run_bass_kernel_spmd(nc, in_maps, list(range(N_CORES)),
                               trace=_trace)

    inv = np.argsort(np.array(PERM_LM))  # lm -> row-block index
    out = np.zeros((NNODES, 16, NCHAN), np.float32)
    for c in range(N_CORES):
        slots = res.results[c]["out"].astype(np.float32)  # [128, T, 64]
        slots = slots.reshape(16, 8, -1, NCHAN)[inv]  # [lm, k, T, c]
        smap = slot_maps[c]
        for t, (n0, k) in enumerate(smap):
            if k == 0:
                continue
            out[n0:n0 + k] += slots[:, 0:k, t, :].transpose(1, 0, 2)
    if _trace:
        return out, res
    return out
